# revision 12
# baseline (speedup 1.0000x reference)
"""Causal self-attention Trainium2 kernel (B=8, S=1024, C=768, H=12).

Sharding: pure data-parallel over batch — core i computes batch i end-to-end.
No collectives. Weights are replicated to all 8 cores.

v3 structure (per core, batch b):
  xT        [C, S]  (host-transposed slice of x)
  K/Q proj  qk[c', s], per-ct interleaved DMA so the first matmul starts
            as soon as the first x/w tiles land
  V proj    vp[s, h, 65] (ones column fused for softmax denominator)
  attention is k-tile-major over two q-halves (512 wide):
    for each head pair and k-tile j: logits [128, span] with span =
    512 - max(0, j*128 - q0) — head pair row-tiled on the PE array
    (K=64 at rows 0-63 / 64-127 run concurrently via tile_position);
    exp on ScalarE; triangular mask multiply on the diagonal 128 cols;
    AV accumulates into psum[65, 512] (row 64 = denominator).
  norm      reciprocal_approx_fast on PSUM row 64 -> partition_broadcast
            -> fused evacuate-multiply into y[c, s]
  out-proj  per finished s-tile, woven between attention pairs
"""

import sys
import types

import numpy as np

import concourse.bass as bass
import concourse.mybir as mybir
import concourse.tile as tile
from concourse import bacc
from concourse.masks import make_upper_triangular


def _ensure_axon_hooks():
    """The container's `antenv` stub lacks `axon_hooks`, which
    run_bass_kernel_spmd imports when trace=True under axon. Provide it and
    register the NTFF profile hook so tracing works."""
    try:
        import antenv.axon_hooks  # noqa: F401

        return
    except ImportError:
        pass
    try:
        import antenv
    except ImportError:
        return
    mod = types.ModuleType("antenv.axon_hooks")
    _store = [None]
    mod.set_axon_ntff_profile_hook = lambda h: _store.__setitem__(0, h)
    mod.get_axon_ntff_profile_hook = lambda: _store[0]
    sys.modules["antenv.axon_hooks"] = mod
    antenv.axon_hooks = mod
    try:
        from trn_agent_boot.trn_boot import _ntff_profile_via_ctypes

        hook = _ntff_profile_via_ctypes("/opt/axon/libaxon_pjrt.so")
        mod.set_axon_ntff_profile_hook(hook)
    except Exception:
        pass


_ensure_axon_hooks()

P = 128
C = 768
H = 12
D = 64
NT_C = C // P          # 6 c-tiles
HB = 512               # q-half width
F32 = mybir.dt.float32
F16 = mybir.dt.float16
EXPF = mybir.ActivationFunctionType.Exp


def build_nc(S=1024):
    NT_S = S // P          # 8 s-tiles
    NH = S // HB           # 2 q-halves

    nc = bacc.Bacc("TRN2", target_bir_lowering=False, debug=False)

    xt_d = nc.dram_tensor("xt", [C, S], F16, kind="ExternalInput")
    wqk_d = nc.dram_tensor("wqkT", [C, 2 * C], F16, kind="ExternalInput")
    wv_d = nc.dram_tensor("wvT", [C, C], F16, kind="ExternalInput")
    wo_d = nc.dram_tensor("woutT", [C, C], F16, kind="ExternalInput")
    bqk_d = nc.dram_tensor("bqk", [2 * C], F32, kind="ExternalInput")
    bv_d = nc.dram_tensor("bv", [C], F32, kind="ExternalInput")
    bo_d = nc.dram_tensor("bout", [C], F32, kind="ExternalInput")
    out_d = nc.dram_tensor("out", [S, C], F32, kind="ExternalOutput")

    with tile.TileContext(nc) as tc:
        with (
            tc.tile_pool(name="const", bufs=1) as cpool,
            tc.tile_pool(name="big", bufs=1) as gpool,
            tc.tile_pool(name="ptile", bufs=6) as ppool,
            tc.tile_pool(name="evac", bufs=2) as epool,
            tc.tile_pool(name="recip", bufs=4) as rpool,
            tc.tile_pool(name="bcast", bufs=4) as bpool,
            tc.tile_pool(name="proj_ps", bufs=2, space="PSUM") as proj_ps,
            tc.tile_pool(name="logit_ps", bufs=3, space="PSUM") as logit_ps,
            tc.tile_pool(name="av_ps", bufs=3, space="PSUM") as av_ps,
        ):
            # ------------- persistent SBUF tensors, interleaved DMA -------------
            # V weights + x per ct-tile so V-proj fills the DMA head window.
            xt_sb = gpool.tile([P, NT_C, S], F16)
            wqk_sb = gpool.tile([P, NT_C, 2 * C], F16)
            wv_sb = gpool.tile([P, NT_C, C], F16)
            xt_r = xt_d[:, :].rearrange("(ct p) s -> p ct s", p=P)
            wqk_r = wqk_d[:, :].rearrange("(ct p) n -> p ct n", p=P)
            wv_r = wv_d[:, :].rearrange("(ct p) n -> p ct n", p=P)
            for ct in range(NT_C):
                nc.sync.dma_start(xt_sb[:, ct, :], xt_r[:, ct, :])
                nc.sync.dma_start(wv_sb[:, ct, :], wv_r[:, ct, :])
            nc.sync.dma_start(wqk_sb[:, :, C : 2 * C], wqk_r[:, :, C : 2 * C])
            nc.sync.dma_start(wqk_sb[:, :, 0:C], wqk_r[:, :, 0:C])
            wo_sb = gpool.tile([P, NT_C, C], F16)
            nc.sync.dma_start(wo_sb[:], wo_d[:, :].rearrange("(ct p) n -> p ct n", p=P))

            # ---------------- constants ----------------
            trimask = cpool.tile([P, P], F16)      # 1.0 where p <= f else 0.0
            make_upper_triangular(nc, trimask[:], val=1.0, diag=True)

            bqk_sb = cpool.tile([P, 2 * NT_C], F32)
            nc.scalar.dma_start(bqk_sb[:], bqk_d[:].rearrange("(t p) -> p t", p=P))
            bv_bc = cpool.tile([P, C], F32)
            nc.scalar.dma_start(bv_bc[:], bv_d[:][None, :].to_broadcast((P, C)))
            bo_bc = cpool.tile([P, C], F32)
            nc.scalar.dma_start(bo_bc[:], bo_d[:][None, :].to_broadcast((P, C)))

            qk_sb = gpool.tile([P, 2 * NT_C, S], F16)   # Q tiles 0..5, K tiles 6..11
            vp_sb = gpool.tile([P, NT_S, H, D + 1], F16)  # [s, st, h, d|1]
            nc.vector.memset(vp_sb[:, :, :, D : D + 1], 1.0)
            y_sb = gpool.tile([P, NT_C, S], F16)

            # ---------------- K then Q projection: qk[c', s] ----------------
            def proj_qk(t):
                for sb in range(S // 512):
                    ss = slice(sb * 512, (sb + 1) * 512)
                    ps = proj_ps.tile([P, 512], F32, tag="proj", name=f"ps_qk{t}_{sb}")
                    for ct in range(NT_C):
                        nc.tensor.matmul(
                            ps[:],
                            wqk_sb[:, ct, t * P : (t + 1) * P],
                            xt_sb[:, ct, ss],
                            start=(ct == 0),
                            stop=(ct == NT_C - 1),
                        )
                    nc.vector.tensor_scalar_add(
                        qk_sb[:, t, ss], ps[:], bqk_sb[:, t : t + 1]
                    )

            # ---------------- V projection: vp[s, h, d] + ones col ----------
            def proj_v_chunk(st, ci):
                cs, cw = ((0, 512), (512, 256))[ci]
                ps = proj_ps.tile([P, 512], F32, tag="proj", name=f"ps_v{st}_{ci}")
                for ct in range(NT_C):
                    nc.tensor.matmul(
                        ps[:, :cw],
                        xt_sb[:, ct, st * P : (st + 1) * P],
                        wv_sb[:, ct, cs : cs + cw],
                        start=(ct == 0),
                        stop=(ct == NT_C - 1),
                    )
                nh = cw // D
                h0 = cs // D
                nc.vector.tensor_add(
                    vp_sb[:, st, h0 : h0 + nh, 0:D],
                    ps[:, :cw].rearrange("p (h d) -> p h d", d=D),
                    bv_bc[:, cs : cs + cw].rearrange("p (h d) -> p h d", d=D),
                )

            # ---------------- out-projection for one s-tile ----------------
            ot_tiles = {}

            def proj_out_chunk(st, ci):
                cs, cw = ((0, 512), (512, 256))[ci]
                if ci == 0:
                    ot_tiles[st] = epool.tile([P, C], F32, tag="ot", name=f"ot{st}")
                ot = ot_tiles[st]
                ps = proj_ps.tile([P, 512], F32, tag="proj", name=f"ps_o{st}_{ci}")
                for ct in range(NT_C):
                    nc.tensor.matmul(
                        ps[:, :cw],
                        y_sb[:, ct, st * P : (st + 1) * P],
                        wo_sb[:, ct, cs : cs + cw],
                        start=(ct == 0),
                        stop=(ct == NT_C - 1),
                    )
                nc.vector.tensor_add(
                    ot[:, cs : cs + cw], ps[:, :cw], bo_bc[:, cs : cs + cw]
                )
                if ci == 1:
                    nc.sync.dma_start(out_d[st * P : (st + 1) * P, :], ot[:])

            for st in range(6):               # V tiles fill the DMA head
                proj_v_chunk(st, 0)
                proj_v_chunk(st, 1)
            for t in range(NT_C, 2 * NT_C):   # K tiles
                proj_qk(t)
            for t in range(NT_C):             # Q tiles
                proj_qk(t)

            # ---------------- attention, k-tile-major over q-halves ----------
            # weave[half][pair] -> list of thunks issued after that pair, to
            # keep the PE array dense across pair boundaries (HAM clock gate)
            weave = {0: {}, 1: {}}
            weave[0][0] = [lambda: proj_v_chunk(6, 0), lambda: proj_v_chunk(6, 1)]
            weave[0][1] = [lambda: proj_v_chunk(7, 0), lambda: proj_v_chunk(7, 1)]
            for pr in range(4):
                weave[1][pr] = [
                    lambda st=pr: proj_out_chunk(st, 0),
                    lambda st=pr: proj_out_chunk(st, 1),
                ]

            for half in range(NH):
                q0 = half * HB
                jmax = (q0 + HB) // P          # k-tiles 0..jmax-1
                hs = slice(q0, q0 + HB)
                for pair in range(NT_C):
                    kt = NT_C + pair
                    pts = {}
                    avs = {}
                    for hh in range(2):
                        avs[hh] = av_ps.tile(
                            [D + 1, HB], F32, tag="av", name=f"av{half}_{pair}_{hh}"
                        )

                    def qk_exp_step(j):
                        """Logits (head pair row-tiled) + exp + diagonal mask."""
                        qlo = max(0, j * P - q0)
                        lgs = {}
                        for hh in range(2):
                            lo = hh * D
                            lg = logit_ps.tile(
                                [P, HB], F32, tag="lg",
                                name=f"lg{half}_{pair}_{j}_{hh}",
                            )
                            nc.tensor.matmul(
                                lg[:, qlo:HB],
                                qk_sb[lo : lo + D, kt, j * P : (j + 1) * P],
                                qk_sb[lo : lo + D, pair, q0 + qlo : q0 + HB],
                                start=True,
                                stop=True,
                                skip_group_check=True,
                                tile_position=(lo, 0),
                            )
                            lgs[hh] = lg
                        for hh in range(2):
                            pt = ppool.tile(
                                [P, HB], F16, tag="pt",
                                name=f"pt{half}_{pair}_{j}_{hh}",
                            )
                            nc.scalar.activation(
                                pt[:, qlo:HB], lgs[hh][:, qlo:HB], EXPF, scale=0.125
                            )
                            if j * P >= q0:   # diagonal k-tile: mask 128 cols
                                nc.vector.tensor_mul(
                                    pt[:, qlo : qlo + P],
                                    pt[:, qlo : qlo + P],
                                    trimask[:],
                                )
                            pts[(j, hh)] = pt

                    def av_step(j):
                        """AV accumulation for k-tile j (row 64 = denominator)."""
                        qlo = max(0, j * P - q0)
                        for hh in range(2):
                            h = 2 * pair + hh
                            nc.tensor.matmul(
                                avs[hh][:, qlo:HB],
                                vp_sb[:, j, h, :],
                                pts[(j, hh)][:, qlo:HB],
                                start=(j == 0),
                                stop=(j == jmax - 1),
                                skip_group_check=True,
                            )

                    # software pipeline: QK/exp one k-tile ahead of AV
                    qk_exp_step(0)
                    for j in range(1, jmax):
                        qk_exp_step(j)
                        av_step(j - 1)
                    av_step(jmax - 1)
                    # --- normalize + evacuate into y[c, s] ---
                    for hh in range(2):
                        lo2 = hh * D
                        avp = avs[hh]
                        dsb = rpool.tile(
                            [1, HB], F32, tag="dsb", name=f"dsb{half}_{pair}_{hh}"
                        )
                        nc.vector.tensor_copy(dsb[:], avp[D : D + 1, :])
                        rc = rpool.tile(
                            [1, HB], F32, tag="rc", name=f"rc{half}_{pair}_{hh}"
                        )
                        nc.vector.reciprocal_approx_fast(rc[:], dsb[:])
                        bc = bpool.tile(
                            [D, HB], F32, tag="bc", name=f"bc{half}_{pair}_{hh}"
                        )
                        nc.gpsimd.partition_broadcast(bc[:], rc[:])
                        nc.vector.tensor_mul(
                            y_sb[lo2 : lo2 + D, pair, hs], avp[0:D, :], bc[:]
                        )
                    for thunk in weave[half].get(pair, ()):
                        thunk()
                # out-projection for the last finished q-half
                if half == NH - 1:
                    for st in range(4 * half, 4 * half + 4):
                        proj_out_chunk(st, 0)
                        proj_out_chunk(st, 1)

    nc.compile()
    return nc


_NC_CACHE = {}


def _get_nc(S):
    if S not in _NC_CACHE:
        _NC_CACHE[S] = build_nc(S)
    return _NC_CACHE[S]


def make_in_maps(x, w_qkv, b_qkv, w_out, b_out):
    x = np.asarray(x, np.float32)
    w_qkv = np.asarray(w_qkv, np.float32)
    b_qkv = np.asarray(b_qkv, np.float32)
    w_out = np.asarray(w_out, np.float32)
    b_out = np.asarray(b_out, np.float32)
    B = x.shape[0]
    xt = np.ascontiguousarray(x.transpose(0, 2, 1)).astype(np.float16)
    wqkT = np.ascontiguousarray(w_qkv[: 2 * C].T).astype(np.float16)
    wvT = np.ascontiguousarray(w_qkv[2 * C :].T).astype(np.float16)
    woT = np.ascontiguousarray(w_out.T).astype(np.float16)
    bqk = np.ascontiguousarray(b_qkv[: 2 * C])
    bv = np.ascontiguousarray(b_qkv[2 * C :])
    bo = np.ascontiguousarray(b_out)
    return [
        {
            "xt": xt[i],
            "wqkT": wqkT,
            "wvT": wvT,
            "woutT": woT,
            "bqk": bqk,
            "bv": bv,
            "bout": bo,
        }
        for i in range(B)
    ]


def kernel_with_results(x, w_qkv, b_qkv, w_out, b_out, attention_mask=None, **run_kw):
    from concourse.bass_utils import run_bass_kernel_spmd

    B, S, C_ = x.shape
    assert C_ == C
    nc = _get_nc(S)
    in_maps = make_in_maps(x, w_qkv, b_qkv, w_out, b_out)
    res = run_bass_kernel_spmd(nc, in_maps, core_ids=list(range(B)), **run_kw)
    out = np.stack([m["out"] for m in res.results], axis=0).astype(np.float32)
    return out, res


def kernel(x, w_qkv, b_qkv, w_out, b_out, attention_mask=None):
    out, _ = kernel_with_results(x, w_qkv, b_qkv, w_out, b_out, attention_mask)
    return out


# revision 18
# speedup vs baseline: 1.0604x; 1.0604x over previous
"""Causal self-attention Trainium2 kernel (B=8, S=1024, C=768, H=12).

Sharding: pure data-parallel over batch — core i computes batch i end-to-end.
No collectives. Weights are replicated to all 8 cores.

v3 structure (per core, batch b):
  xT        [C, S]  (host-transposed slice of x)
  K/Q proj  qk[c', s], per-ct interleaved DMA so the first matmul starts
            as soon as the first x/w tiles land
  V proj    vp[s, h, 65] (ones column fused for softmax denominator)
  attention is k-tile-major over two q-halves (512 wide):
    for each head pair and k-tile j: logits [128, span] with span =
    512 - max(0, j*128 - q0) — head pair row-tiled on the PE array
    (K=64 at rows 0-63 / 64-127 run concurrently via tile_position);
    exp on ScalarE; triangular mask multiply on the diagonal 128 cols;
    AV accumulates into psum[65, 512] (row 64 = denominator).
  norm      reciprocal_approx_fast on PSUM row 64 -> partition_broadcast
            -> fused evacuate-multiply into y[c, s]
  out-proj  per finished s-tile, woven between attention pairs
"""

import sys
import types

import numpy as np

import concourse.bass as bass
import concourse.mybir as mybir
import concourse.tile as tile
from concourse import bacc
from concourse.masks import make_upper_triangular


def _ensure_axon_hooks():
    """The container's `antenv` stub lacks `axon_hooks`, which
    run_bass_kernel_spmd imports when trace=True under axon. Provide it and
    register the NTFF profile hook so tracing works."""
    try:
        import antenv.axon_hooks  # noqa: F401

        return
    except ImportError:
        pass
    try:
        import antenv
    except ImportError:
        return
    mod = types.ModuleType("antenv.axon_hooks")
    _store = [None]
    mod.set_axon_ntff_profile_hook = lambda h: _store.__setitem__(0, h)
    mod.get_axon_ntff_profile_hook = lambda: _store[0]
    sys.modules["antenv.axon_hooks"] = mod
    antenv.axon_hooks = mod
    try:
        from trn_agent_boot.trn_boot import _ntff_profile_via_ctypes

        hook = _ntff_profile_via_ctypes("/opt/axon/libaxon_pjrt.so")
        mod.set_axon_ntff_profile_hook(hook)
    except Exception:
        pass


_ensure_axon_hooks()

P = 128
C = 768
H = 12
D = 64
NT_C = C // P          # 6 c-tiles
HB = 512               # q-half width
F32 = mybir.dt.float32
F16 = mybir.dt.float16
EXPF = mybir.ActivationFunctionType.Exp


def build_nc(S=1024):
    NT_S = S // P          # 8 s-tiles
    NH = S // HB           # 2 q-halves

    nc = bacc.Bacc("TRN2", target_bir_lowering=False, debug=False)

    xt_d = nc.dram_tensor("xt", [C, S], F16, kind="ExternalInput")
    # wqk tile-major: [t, p, ct*128] so each feature tile is one efficient DMA
    wqk_d = nc.dram_tensor("wqkt", [2 * NT_C, P, C], F16, kind="ExternalInput")
    wv_d = nc.dram_tensor("wvT", [C, C], F16, kind="ExternalInput")
    wo_d = nc.dram_tensor("woutT", [C, C], F16, kind="ExternalInput")
    bqk_d = nc.dram_tensor("bqk", [2 * C], F32, kind="ExternalInput")
    bv_d = nc.dram_tensor("bv", [C], F32, kind="ExternalInput")
    bo_d = nc.dram_tensor("bout", [C], F32, kind="ExternalInput")
    out_d = nc.dram_tensor("out", [S, C], F32, kind="ExternalOutput")

    with tile.TileContext(nc) as tc:
        with (
            tc.tile_pool(name="const", bufs=1) as cpool,
            tc.tile_pool(name="big", bufs=1) as gpool,
            tc.tile_pool(name="ptile", bufs=6) as ppool,
            tc.tile_pool(name="evac", bufs=2) as epool,
            tc.tile_pool(name="recip", bufs=4) as rpool,
            tc.tile_pool(name="bcast", bufs=4) as bpool,
            tc.tile_pool(name="proj_ps", bufs=2, space="PSUM") as proj_ps,
            tc.tile_pool(name="logit_ps", bufs=3, space="PSUM") as logit_ps,
            tc.tile_pool(name="av_ps", bufs=3, space="PSUM") as av_ps,
        ):
            # ------------- persistent SBUF tensors, staged DMA -------------
            # Order: x, then the K/Q weight tiles needed by the first
            # attention pair, then V weights, then the remaining K/Q tiles
            # pair by pair, then the out-proj weights (needed last).
            xt_sb = gpool.tile([P, NT_C, S], F16)
            wqk_sb = gpool.tile([P, 2 * NT_C, NT_C, P], F16)  # [p, t, ct, n]
            wv_sb = gpool.tile([P, NT_C, C], F16)
            xt_r = xt_d[:, :].rearrange("(ct p) s -> p ct s", p=P)
            wv_r = wv_d[:, :].rearrange("(ct p) n -> p ct n", p=P)
            wqk_r = wqk_d[:, :, :].rearrange("t p (ct n) -> t p ct n", n=P)

            def dma_wqk(t):
                nc.sync.dma_start(wqk_sb[:, t, :, :], wqk_r[t])

            for ct in range(NT_C):
                nc.sync.dma_start(xt_sb[:, ct, :], xt_r[:, ct, :])
            dma_wqk(NT_C)      # K tile for pair 0
            dma_wqk(0)         # Q tile for pair 0
            nc.sync.dma_start(wv_sb[:], wv_r)
            for pr in range(1, NT_C):
                dma_wqk(NT_C + pr)
                dma_wqk(pr)
            wo_sb = gpool.tile([P, NT_C, C], F16)
            nc.sync.dma_start(wo_sb[:], wo_d[:, :].rearrange("(ct p) n -> p ct n", p=P))

            # ---------------- constants ----------------
            trimask = cpool.tile([P, P], F16)      # 1.0 where p <= f else 0.0
            make_upper_triangular(nc, trimask[:], val=1.0, diag=True)

            bqk_sb = cpool.tile([P, 2 * NT_C], F32)
            nc.scalar.dma_start(bqk_sb[:], bqk_d[:].rearrange("(t p) -> p t", p=P))
            bv_bc = cpool.tile([P, C], F32)
            nc.scalar.dma_start(bv_bc[:], bv_d[:][None, :].to_broadcast((P, C)))
            bo_bc = cpool.tile([P, C], F32)
            nc.scalar.dma_start(bo_bc[:], bo_d[:][None, :].to_broadcast((P, C)))

            qk_sb = gpool.tile([P, 2 * NT_C, S], F16)   # Q tiles 0..5, K tiles 6..11
            vp_sb = gpool.tile([P, NT_S, H, D + 1], F16)  # [s, st, h, d|1]
            nc.vector.memset(vp_sb[:, :, :, D : D + 1], 1.0)
            y_sb = gpool.tile([P, NT_C, S], F16)

            # ---------------- K/Q projection: qk[c', s] ----------------
            def proj_qk(t):
                for sb in range(S // 512):
                    ss = slice(sb * 512, (sb + 1) * 512)
                    ps = proj_ps.tile([P, 512], F32, tag="proj", name=f"ps_qk{t}_{sb}")
                    for ct in range(NT_C):
                        nc.tensor.matmul(
                            ps[:],
                            wqk_sb[:, t, ct, :],
                            xt_sb[:, ct, ss],
                            start=(ct == 0),
                            stop=(ct == NT_C - 1),
                        )
                    nc.vector.tensor_scalar_add(
                        qk_sb[:, t, ss], ps[:], bqk_sb[:, t : t + 1]
                    )

            # ---------------- V projection: vp[s, h, d] + ones col ----------
            def proj_v_chunk(st, ci):
                cs, cw = ((0, 512), (512, 256))[ci]
                ps = proj_ps.tile([P, 512], F32, tag="proj", name=f"ps_v{st}_{ci}")
                for ct in range(NT_C):
                    nc.tensor.matmul(
                        ps[:, :cw],
                        xt_sb[:, ct, st * P : (st + 1) * P],
                        wv_sb[:, ct, cs : cs + cw],
                        start=(ct == 0),
                        stop=(ct == NT_C - 1),
                    )
                nh = cw // D
                h0 = cs // D
                nc.vector.tensor_add(
                    vp_sb[:, st, h0 : h0 + nh, 0:D],
                    ps[:, :cw].rearrange("p (h d) -> p h d", d=D),
                    bv_bc[:, cs : cs + cw].rearrange("p (h d) -> p h d", d=D),
                )

            # ---------------- out-projection for one s-tile ----------------
            ot_tiles = {}

            def proj_out_chunk(st, ci):
                cs, cw = ((0, 512), (512, 256))[ci]
                if ci == 0:
                    ot_tiles[st] = epool.tile([P, C], F32, tag="ot", name=f"ot{st}")
                ot = ot_tiles[st]
                ps = proj_ps.tile([P, 512], F32, tag="proj", name=f"ps_o{st}_{ci}")
                for ct in range(NT_C):
                    nc.tensor.matmul(
                        ps[:, :cw],
                        y_sb[:, ct, st * P : (st + 1) * P],
                        wo_sb[:, ct, cs : cs + cw],
                        start=(ct == 0),
                        stop=(ct == NT_C - 1),
                    )
                nc.vector.tensor_add(
                    ot[:, cs : cs + cw], ps[:, :cw], bo_bc[:, cs : cs + cw]
                )
                if ci == 1:
                    nc.sync.dma_start(out_d[st * P : (st + 1) * P, :], ot[:])

            # Minimal prefix for attention pair 0 of q-half 0:
            proj_qk(NT_C)                     # K tile 6
            proj_qk(0)                        # Q tile 0
            for st in range(4):               # V tiles for q-half 0
                proj_v_chunk(st, 0)
                proj_v_chunk(st, 1)

            # ---------------- attention, k-tile-major over q-halves ----------
            # weave[half][pair] -> thunks issued after that pair. The rest of
            # the K/Q/V projections ride inside q-half 0 (keeping the PE dense
            # and the exp stream fed); out-proj rides inside q-half 1.
            weave = {0: {}, 1: {}}
            for pr in range(5):
                weave[0][pr] = [
                    lambda t=NT_C + 1 + pr: proj_qk(t),   # K tile for pair pr+1
                    lambda t=1 + pr: proj_qk(t),          # Q tile for pair pr+1
                ]
            for pr in range(4):
                weave[0][1 + pr] += [
                    lambda st=4 + pr: proj_v_chunk(st, 0),
                    lambda st=4 + pr: proj_v_chunk(st, 1),
                ]
            for pr in range(4):
                weave[1][pr] = [
                    lambda st=pr: proj_out_chunk(st, 0),
                    lambda st=pr: proj_out_chunk(st, 1),
                ]

            for half in range(NH):
                q0 = half * HB
                jmax = (q0 + HB) // P          # k-tiles 0..jmax-1
                hs = slice(q0, q0 + HB)
                for pair in range(NT_C):
                    kt = NT_C + pair
                    pts = {}
                    avs = {}
                    for hh in range(2):
                        avs[hh] = av_ps.tile(
                            [D + 1, HB], F32, tag="av", name=f"av{half}_{pair}_{hh}"
                        )

                    def qk_exp_step(j):
                        """Logits (head pair row-tiled) + exp + diagonal mask."""
                        qlo = max(0, j * P - q0)
                        lgs = {}
                        for hh in range(2):
                            lo = hh * D
                            lg = logit_ps.tile(
                                [P, HB], F32, tag="lg",
                                name=f"lg{half}_{pair}_{j}_{hh}",
                            )
                            nc.tensor.matmul(
                                lg[:, qlo:HB],
                                qk_sb[lo : lo + D, kt, j * P : (j + 1) * P],
                                qk_sb[lo : lo + D, pair, q0 + qlo : q0 + HB],
                                start=True,
                                stop=True,
                                skip_group_check=True,
                                tile_position=(lo, 0),
                            )
                            lgs[hh] = lg
                        for hh in range(2):
                            pt = ppool.tile(
                                [P, HB], F16, tag="pt",
                                name=f"pt{half}_{pair}_{j}_{hh}",
                            )
                            nc.scalar.activation(
                                pt[:, qlo:HB], lgs[hh][:, qlo:HB], EXPF, scale=0.125
                            )
                            if j * P >= q0:   # diagonal k-tile: mask 128 cols
                                nc.vector.tensor_mul(
                                    pt[:, qlo : qlo + P],
                                    pt[:, qlo : qlo + P],
                                    trimask[:],
                                )
                            pts[(j, hh)] = pt

                    def av_step(j):
                        """AV accumulation for k-tile j (row 64 = denominator)."""
                        qlo = max(0, j * P - q0)
                        for hh in range(2):
                            h = 2 * pair + hh
                            nc.tensor.matmul(
                                avs[hh][:, qlo:HB],
                                vp_sb[:, j, h, :],
                                pts[(j, hh)][:, qlo:HB],
                                start=(j == 0),
                                stop=(j == jmax - 1),
                                skip_group_check=True,
                            )

                    # software pipeline: QK/exp one k-tile ahead of AV
                    qk_exp_step(0)
                    for j in range(1, jmax):
                        qk_exp_step(j)
                        av_step(j - 1)
                    av_step(jmax - 1)
                    # --- normalize + evacuate into y[c, s] ---
                    for hh in range(2):
                        lo2 = hh * D
                        avp = avs[hh]
                        dsb = rpool.tile(
                            [1, HB], F32, tag="dsb", name=f"dsb{half}_{pair}_{hh}"
                        )
                        nc.vector.tensor_copy(dsb[:], avp[D : D + 1, :])
                        rc = rpool.tile(
                            [1, HB], F32, tag="rc", name=f"rc{half}_{pair}_{hh}"
                        )
                        nc.vector.reciprocal_approx_fast(rc[:], dsb[:])
                        bc = bpool.tile(
                            [D, HB], F32, tag="bc", name=f"bc{half}_{pair}_{hh}"
                        )
                        nc.gpsimd.partition_broadcast(bc[:], rc[:])
                        nc.vector.tensor_mul(
                            y_sb[lo2 : lo2 + D, pair, hs], avp[0:D, :], bc[:]
                        )
                    for thunk in weave[half].get(pair, ()):
                        thunk()
                # out-projection for the last finished q-half
                if half == NH - 1:
                    for st in range(4 * half, 4 * half + 4):
                        proj_out_chunk(st, 0)
                        proj_out_chunk(st, 1)

    nc.compile()
    return nc


_NC_CACHE = {}


def _get_nc(S):
    if S not in _NC_CACHE:
        _NC_CACHE[S] = build_nc(S)
    return _NC_CACHE[S]


def make_in_maps(x, w_qkv, b_qkv, w_out, b_out):
    x = np.asarray(x, np.float32)
    w_qkv = np.asarray(w_qkv, np.float32)
    b_qkv = np.asarray(b_qkv, np.float32)
    w_out = np.asarray(w_out, np.float32)
    b_out = np.asarray(b_out, np.float32)
    B = x.shape[0]
    xt = np.ascontiguousarray(x.transpose(0, 2, 1)).astype(np.float16)
    wqkT = w_qkv[: 2 * C].T  # [c, n]
    # tile-major: [t, p, ct*128+nn] = wqkT[ct*128+p, t*128+nn]
    wqkt = np.ascontiguousarray(
        wqkT.reshape(NT_C, P, 2 * NT_C, P).transpose(2, 1, 0, 3).reshape(
            2 * NT_C, P, C
        )
    ).astype(np.float16)
    wvT = np.ascontiguousarray(w_qkv[2 * C :].T).astype(np.float16)
    woT = np.ascontiguousarray(w_out.T).astype(np.float16)
    bqk = np.ascontiguousarray(b_qkv[: 2 * C])
    bv = np.ascontiguousarray(b_qkv[2 * C :])
    bo = np.ascontiguousarray(b_out)
    return [
        {
            "xt": xt[i],
            "wqkt": wqkt,
            "wvT": wvT,
            "woutT": woT,
            "bqk": bqk,
            "bv": bv,
            "bout": bo,
        }
        for i in range(B)
    ]


def kernel_with_results(x, w_qkv, b_qkv, w_out, b_out, attention_mask=None, **run_kw):
    from concourse.bass_utils import run_bass_kernel_spmd

    B, S, C_ = x.shape
    assert C_ == C
    nc = _get_nc(S)
    in_maps = make_in_maps(x, w_qkv, b_qkv, w_out, b_out)
    res = run_bass_kernel_spmd(nc, in_maps, core_ids=list(range(B)), **run_kw)
    out = np.stack([m["out"] for m in res.results], axis=0).astype(np.float32)
    return out, res


def kernel(x, w_qkv, b_qkv, w_out, b_out, attention_mask=None):
    out, _ = kernel_with_results(x, w_qkv, b_qkv, w_out, b_out, attention_mask)
    return out
